# revision 9
# baseline (speedup 1.0000x reference)
"""Trainium2 Bass kernel for nn_Attention_25769804179 — pipelined stage-2.

Multi-head attention (B=4, S=2048, D=1024, H=16, hd=64), fp32 I/O.
Sharding: batch (4-way) x head-group (2-way, 8 heads each) over 8 NeuronCores.

Design (v2): stage-2 is ACT(exp)-bound (~294us/iter: 256 exp tiles of
[128,1024] at (N+352)/1.2 ns). The kernel is organized so the ACT engine
streams exp back-to-back while the PE runs one k-step AHEAD on scores and one
k-step BEHIND on pv, with stage-1/stage-3/normalize work drip-fed as
cost-budgeted fillers in the remaining PE slack:

  step t:   ACT: exp(t)        <- sc(t) emitted on PE during step t-1
            PE:  sc(t+1)       (2 row-split concurrent MMs, K=64)
            PE:  pv(t-1)       (2 serial MMs, K=128; ex(t-1) ready)
            PE:  fillers       (~0.5us budget: stage1/stage3/bc thunks)

The 1/Z normalize chain runs on DVE right after the last pv of a block; the
broadcast matmul (single K=2 MM via a 2-row selector) + ctxt multiply are
deferred into the filler queue so the PE never stalls on the DVE chain.
"""

import sys
import time
from collections import deque

sys.path.insert(0, "/opt/trn_rl_repo")

import numpy as np

B, S, D = 4, 2048, 1024
NH, HD = 16, 64
HPC = 8          # heads per core
NPAIR = HPC // 2
SCALE = HD ** -0.5
NKT = S // 128   # 16 k-tiles
NSQ = S // 512   # 4 q-blocks of 512
NDT = D // 128   # 8 d-tiles
NCORES = 8

MM_NS = 427          # PE cost of one filler mm-thunk (2 x N=512 matmuls)
BC_NS = 217
FILL_BUDGET = 500    # target filler PE-ns per stage-2 step
FILL_BANK_CAP = 700

import os
PROBE = os.environ.get("KERNEL2_PROBE", "")  # "", "stage2", "noends"


def _build(repeat=1):
    import concourse.mybir as mybir
    import concourse.tile as tile
    from concourse import bacc

    dt = mybir.dt
    f32, f16 = dt.float32, dt.float16

    nc = bacc.Bacc("TRN2", debug=False, enable_partition_id=False)

    env = {}
    env["xt_d"] = nc.dram_tensor("xt", [NSQ, 128, NDT * 512], f16, kind="ExternalInput").ap()
    env["wqk_d"] = nc.dram_tensor("wqk", [2, NPAIR, 128, NDT * 128], f16, kind="ExternalInput").ap()
    env["wv_d"] = nc.dram_tensor("wv", [128, NDT * 512], f16, kind="ExternalInput").ap()
    env["wp_d"] = nc.dram_tensor("wp", [128, 4 * 1024], f16, kind="ExternalInput").ap()
    env["bqk_d"] = nc.dram_tensor("bqk", [128, 2 * NPAIR], f32, kind="ExternalInput").ap()
    env["out_d"] = nc.dram_tensor("out", [S // 128, 128, D], f16, kind="ExternalOutput").ap()

    with tile.TileContext(nc) as tc:
        with tc.tile_pool(name="consts", bufs=1) as consts, \
             tc.tile_pool(name="xres", bufs=2) as x_pool, \
             tc.tile_pool(name="vapool", bufs=2) as va_pool, \
             tc.tile_pool(name="wqk", bufs=2) as wqk_pool, \
             tc.tile_pool(name="qt", bufs=3) as qt_pool, \
             tc.tile_pool(name="kt", bufs=3) as kt_pool, \
             tc.tile_pool(name="ex", bufs=3) as ex_pool, \
             tc.tile_pool(name="rc", bufs=2) as rc_pool, \
             tc.tile_pool(name="pvs", bufs=2) as pvs_pool, \
             tc.tile_pool(name="osb", bufs=4) as out_pool, \
             tc.tile_pool(name="ps1", bufs=2, space="PSUM") as ps1, \
             tc.tile_pool(name="pssc", bufs=2, space="PSUM") as ps_sc, \
             tc.tile_pool(name="pspv", bufs=1, space="PSUM") as ps_pv:

            env.update(x_pool=x_pool, va_pool=va_pool, wqk_pool=wqk_pool,
                       qt_pool=qt_pool, kt_pool=kt_pool, ex_pool=ex_pool,
                       rc_pool=rc_pool, pvs_pool=pvs_pool, out_pool=out_pool,
                       ps1=ps1, ps_sc=ps_sc, ps_pv=ps_pv)

            wv_sb = consts.tile([128, NDT * 512], f16, name="wv_sb")
            nc.sync.dma_start(wv_sb[:], env["wv_d"][:])
            wp_sb = consts.tile([128, 4 * 1024], f16, name="wp_sb")
            nc.sync.dma_start(wp_sb[:], env["wp_d"][:])
            bqk_sb = consts.tile([128, 2 * NPAIR], f32, name="bqk_sb")
            nc.sync.dma_start(bqk_sb[:], env["bqk_d"][:])
            ones_h = consts.tile([1, 64], f16, name="ones_h")
            nc.vector.memset(ones_h[:], 1.0)
            wqk_sb = []
            for pp in range(NPAIR):
                wq_c = consts.tile([128, NDT * 128], f16, name=f"wqc_{pp}")
                nc.sync.dma_start(wq_c[:], env["wqk_d"][0, pp])
                wk_c = consts.tile([128, NDT * 128], f16, name=f"wkc_{pp}")
                nc.sync.dma_start(wk_c[:], env["wqk_d"][1, pp])
                wqk_sb.append((wq_c, wk_c))
            env["wqk_sb"] = wqk_sb
            ctxt = consts.tile([128, NPAIR * S], f16, name="ctxt")
            env.update(wv_sb=wv_sb, wp_sb=wp_sb, bqk_sb=bqk_sb, ones_h=ones_h,
                       ctxt=ctxt)

            carry = None
            for r in range(repeat):
                carry = _emit_iter(nc, r, mybir, env, carry,
                                   make_carry=(r + 1 < repeat))

    nc.compile()
    return nc


def _alloc_x(nc, env, rr, f16, chunks=None):
    x_sb = env["x_pool"].tile([128, NSQ * NDT * 512], f16, tag="x", name=f"x_{rr}")
    for c in (range(NSQ) if chunks is None else chunks):
        nc.sync.dma_start(x_sb[:, c * 4096:(c + 1) * 4096], env["xt_d"][c])
    return x_sb


def _alloc_va(nc, env, rr, f16):
    va = env["va_pool"].tile([128, NKT * HPC * 65], f16, tag="va", name=f"va_{rr}")
    vav = va[:].rearrange("p (k c) -> p k c", c=65)
    nc.vector.memset(vav[:, :, 64:65], 1.0)
    return vav


def _emit_iter(nc, r, mybir, env, carry, make_carry):
    """Emit one attention iteration; returns the carry for the next."""
    dt = mybir.dt
    f32, f16 = dt.float32, dt.float16
    AF = mybir.ActivationFunctionType
    ALU = mybir.AluOpType

    wqk_pool = env["wqk_pool"]; qt_pool = env["qt_pool"]; kt_pool = env["kt_pool"]
    ex_pool = env["ex_pool"]; rc_pool = env["rc_pool"]
    pvs_pool = env["pvs_pool"]; out_pool = env["out_pool"]
    ps1 = env["ps1"]; ps_sc = env["ps_sc"]; ps_pv = env["ps_pv"]
    wv_sb = env["wv_sb"]; wp_sb = env["wp_sb"]; bqk_sb = env["bqk_sb"]
    ones_h = env["ones_h"]; ctxt = env["ctxt"]
    wqk_d = env["wqk_d"]; out_d = env["out_d"]

    # ---- stage-1 generators (yield (pe_cost_ns, thunk)) ----

    def gen_qk(rr, p, x_sb, res):
        """QKV projection for pair p's q and k heads; appends (qt,kt) to res."""
        wq_t, wk_t = env["wqk_sb"][p]
        qt_t = qt_pool.tile([128, S], f16, tag="qt", name=f"qt_{rr}_{p}")
        kt_t = kt_pool.tile([128, S], f16, tag="kt", name=f"kt_{rr}_{p}")
        res.append((qt_t, kt_t))
        for c in range(NSQ):
            for qk, (w_t, dst) in enumerate(((wq_t, qt_t), (wk_t, kt_t))):
                ps = ps1.tile([128, 512], f32, tag="ps1", name=f"s1_{rr}_{p}_{c}_{qk}")
                for d0 in range(0, NDT, 2):
                    def mm(d0=d0, ps=ps, w_t=w_t, c=c, x_sb=x_sb):
                        for dtl in (d0, d0 + 1):
                            nc.tensor.matmul(
                                ps[:], w_t[:, dtl * 128:(dtl + 1) * 128],
                                x_sb[:, c * 4096 + dtl * 512: c * 4096 + (dtl + 1) * 512],
                                start=(dtl == 0), stop=(dtl == NDT - 1),
                            )
                    yield (MM_NS, mm)
                def cp(ps=ps, dst=dst, c=c, qk=qk, p=p):
                    nc.vector.tensor_scalar_add(
                        dst[:, c * 512:(c + 1) * 512], ps[:],
                        bqk_sb[:, qk * NPAIR + p: qk * NPAIR + p + 1],
                    )
                yield (0, cp)

    def gen_v(rr, c, x_sb, vav):
        """V projection for q-block c (k-tiles 4c..4c+3), all 8 heads."""
        for ss in range(4):
            ktile = c * 4 + ss
            ps = ps1.tile([128, 512], f32, tag="ps1", name=f"v_{rr}_{c}_{ss}")
            for d0 in range(0, NDT, 2):
                def mm(d0=d0, ps=ps, c=c, ss=ss, x_sb=x_sb):
                    for dtl in (d0, d0 + 1):
                        nc.tensor.matmul(
                            ps[:],
                            x_sb[:, c * 4096 + dtl * 512 + ss * 128:
                                 c * 4096 + dtl * 512 + (ss + 1) * 128],
                            wv_sb[:, dtl * 512:(dtl + 1) * 512],
                            start=(dtl == 0), stop=(dtl == NDT - 1),
                        )
                yield (MM_NS, mm)
            def cp(ps=ps, ktile=ktile, vav=vav):
                nc.vector.tensor_copy(
                    vav[:, ktile * HPC:(ktile + 1) * HPC, 0:64],
                    ps[:].rearrange("p (h e) -> p h e", e=64),
                )
            yield (0, cp)

    def gen_stage3(t):
        """Out projection for q-tile t (128 queries)."""
        o_t = out_pool.tile([128, 1024], f16, tag="o", name=f"o_{r}_{t}")
        for ch in range(2):
            ps = ps1.tile([128, 512], f32, tag="ps1", name=f"s3_{r}_{t}_{ch}")
            for f0 in range(0, NPAIR, 2):
                def mm(f0=f0, ps=ps, ch=ch, t=t):
                    for ft in (f0, f0 + 1):
                        nc.tensor.matmul(
                            ps[:],
                            ctxt[:, ft * S + t * 128: ft * S + (t + 1) * 128],
                            wp_sb[:, ft * 1024 + ch * 512: ft * 1024 + (ch + 1) * 512],
                            start=(ft == 0), stop=(ft == NPAIR - 1),
                        )
                yield (MM_NS, mm)
            def cp(ps=ps, o_t=o_t, ch=ch):
                nc.vector.tensor_copy(o_t[:, ch * 512:(ch + 1) * 512], ps[:])
            yield (0, cp)
        def dma(o_t=o_t, t=t):
            nc.sync.dma_start(out_d[t], o_t[:])
        yield (0, dma)

    def gen_bc(p, j, rc_h, pv_sb):
        """Deferred 1/Z broadcast + ctxt multiply for block (p, j)."""
        def bc(p=p, j=j, rc_h=rc_h, pv_sb=pv_sb):
            bc_ps = ps1.tile([128, 512], f32, tag="ps1", name=f"bc_{r}_{p}_{j}")
            nc.tensor.matmul(bc_ps[0:64, :], ones_h[0:1, :], rc_h[0:1, 0:512])
            nc.tensor.matmul(bc_ps[64:128, :], ones_h[0:1, :], rc_h[0:1, 512:1024])
            cslice = ctxt[:, p * S + j * 512: p * S + (j + 1) * 512]
            nc.vector.tensor_tensor(cslice[0:64, :], bc_ps[0:64, :], pv_sb[0:64, 0:512], ALU.mult)
            nc.vector.tensor_tensor(cslice[64:128, :], bc_ps[64:128, :], pv_sb[0:64, 512:1024], ALU.mult)
        yield (BC_NS, bc)

    # ---- filler queue (inherited from the previous iteration if any) ----
    state = carry["state"] if carry is not None else {
        "fillers": deque(), "pushed": 0, "popped": 0, "bank": 0.0}
    fillers = state["fillers"]

    def push(gen):
        for ent in gen:
            fillers.append(ent)
            state["pushed"] += 1
        return state["pushed"]

    def fill(budget):
        state["bank"] = min(state["bank"] + budget, FILL_BANK_CAP)
        while fillers and state["bank"] > 0:
            cost, th = fillers.popleft()
            th()
            state["popped"] += 1
            state["bank"] -= cost

    def run_until(mark):
        while state["popped"] < mark:
            cost, th = fillers.popleft()
            th()
            state["popped"] += 1

    # ---- pair-0 inputs: from carry, or cold serial prelude ----
    if carry is None:
        x_sb = _alloc_x(nc, env, r, f16)
        vav = _alloc_va(nc, env, r, f16)
        res0 = []
        for _, th in gen_qk(r, 0, x_sb, res0):
            th()
        qt_t, kt_t = res0[0]
        for c in range(NSQ):
            for _, th in gen_v(r, c, x_sb, vav):
                th()
    else:
        x_sb = carry["x_sb"]
        vav = carry["vav"]
        qt_t, kt_t = carry["qtkt"]

    next_carry = None
    next_qt = {}
    marks = {}

    def sc_emit(qt, kt, j, i, p, tag_i):
        sc = ps_sc.tile([128, 1024], f32, tag="sc", name=f"sc_{r}_{p}_{j}_{i}")
        q_sl = qt[:, j * 512:(j + 1) * 512]
        nc.tensor.matmul(sc[:, 0:512], kt[0:64, i * 128:(i + 1) * 128], q_sl[0:64, :])
        nc.tensor.matmul(sc[:, 512:1024], kt[64:128, i * 128:(i + 1) * 128], q_sl[64:128, :])
        return sc

    blocks = [(p, j) for p in range(NPAIR) for j in range(NSQ)]

    # Bootstrap: sc(0) of block 0 (PE; one-step ACT bubble at iteration start)
    sc_cur = sc_emit(qt_t, kt_t, 0, 0, 0, 0)
    pend_pv = deque()    # (closure emitting pv MMs, post_hook or None); lag 2
    pend_s3 = carry["pend_s3"] if carry is not None else []

    for bi, (p, j) in enumerate(blocks):
        # --- per-pair bookkeeping at pair start ---
        if PROBE != "stage2":
            # staggered generator pushes: keep the filler queue shallow so
            # deferred bc/recip/stage-3 thunks pop close to their push point
            if (p, j) == (0, 0):
                res = []
                marks[1] = push(gen_qk(r, 1, x_sb, res))
                next_qt[1] = res
                if make_carry:
                    x_next = _alloc_x(nc, env, r + 1, f16, chunks=[0])
            elif (p, j) == (0, 1) and make_carry:
                nc.sync.dma_start(x_next[:, 1 * 4096:2 * 4096], env["xt_d"][1])
            elif (p, j) == (0, 2):
                res = []
                marks[2] = push(gen_qk(r, 2, x_sb, res))
                next_qt[2] = res
            elif (p, j) == (0, 3) and make_carry:
                nc.sync.dma_start(x_next[:, 2 * 4096:3 * 4096], env["xt_d"][2])
            elif (p, j) == (1, 0) and make_carry:
                nc.sync.dma_start(x_next[:, 3 * 4096:4 * 4096], env["xt_d"][3])
            elif (p, j) == (1, 2):
                res = []
                marks[3] = push(gen_qk(r, 3, x_sb, res))
                next_qt[3] = res
            elif (p, j) == (2, 0) and make_carry:
                res_n = []
                push(gen_qk(r + 1, 0, x_next, res_n))
            elif (p, j) == (2, 2) and make_carry:
                va_next = _alloc_va(nc, env, r + 1, f16)
                push(gen_v(r + 1, 0, x_next, va_next))
                push(gen_v(r + 1, 1, x_next, va_next))
            elif (p, j) == (3, 0) and make_carry:
                push(gen_v(r + 1, 2, x_next, va_next))
                mark = push(gen_v(r + 1, 3, x_next, va_next))
                next_carry = {"x_sb": x_next, "vav": va_next, "qtkt_res": res_n,
                              "mark": mark}

        pv_lo = ps_pv.tile([65, 512], f32, tag="pvlo", name=f"pvl_{r}_{p}_{j}")
        pv_hi = ps_pv.tile([65, 512], f32, tag="pvhi", name=f"pvh_{r}_{p}_{j}")

        for i in range(NKT):
            # 1) ACT: exp of the current sc tile
            ex = ex_pool.tile([128, 1024], f16, tag="ex", name=f"ex_{r}_{p}_{j}_{i}")
            nc.scalar.activation(ex[:], sc_cur[:], AF.Exp, scale=SCALE)

            # 2) PE: look-ahead scores for the next step (possibly next block)
            if i + 1 < NKT:
                sc_next = sc_emit(qt_t, kt_t, j, i + 1, p, i + 1)
            elif bi + 1 < len(blocks):
                np_, nj = blocks[bi + 1]
                if np_ != p and PROBE != "stage2":
                    run_until(marks[np_])
                    qt_n, kt_n = next_qt[np_][0]
                else:
                    qt_n, kt_n = qt_t, kt_t
                sc_next = sc_emit(qt_n, kt_n, nj, 0, np_, 0)
                if np_ != p:
                    qt_t, kt_t = qt_n, kt_n
            else:
                sc_next = None

            # 3) PE: pv, two steps behind (extra slack for the block-boundary
            # pv-bank recycle through the DVE copies)
            if len(pend_pv) == 2:
                pv_th, hook = pend_pv.popleft()
                pv_th()
                if hook is not None:
                    hook()

            def pv_th(i=i, ex=ex, pv_lo=pv_lo, pv_hi=pv_hi, vav=vav, p=p):
                nc.tensor.matmul(
                    pv_lo[:], vav[:, i * HPC + 2 * p, :], ex[:, 0:512],
                    start=(i == 0), stop=(i == NKT - 1),
                )
                nc.tensor.matmul(
                    pv_hi[:], vav[:, i * HPC + 2 * p + 1, :], ex[:, 512:1024],
                    start=(i == 0), stop=(i == NKT - 1),
                )

            hook = None
            if i == NKT - 1 and PROBE != "stage2":
                def hook(p=p, j=j, pv_lo=pv_lo, pv_hi=pv_hi):
                    # stage pv to SBUF now (frees the PSUM bank for the next
                    # block); 1/Z is deferred into the filler queue so the
                    # boundary DVE burst is only 2 ops
                    pv_sb = pvs_pool.tile([65, 1024], f32, tag="pvs", name=f"pvs_{r}_{p}_{j}")
                    nc.vector.tensor_copy(pv_sb[:, 0:512], pv_lo[:])
                    nc.vector.tensor_copy(pv_sb[:, 512:1024], pv_hi[:])
                    rc_h = rc_pool.tile([1, 1024], f16, tag="rch", name=f"rch_{r}_{p}_{j}")
                    def recip(pv_sb=pv_sb, rc_h=rc_h, p=p, j=j):
                        rc_t = rc_pool.tile([1, 1024], f32, tag="rc", name=f"rc_{r}_{p}_{j}")
                        nc.vector.reciprocal(rc_t[0:1, :], pv_sb[64:65, :])
                        nc.vector.tensor_copy(rc_h[:], rc_t[:])
                    if PROBE != "noends":
                        push([(0, recip)])
                        push(gen_bc(p, j, rc_h, pv_sb))
                        for t in (pend_s3.pop(0) if pend_s3 else []):
                            push(gen_stage3(t))
                        if p == NPAIR - 1:
                            # defer one block: stage-3 MMs pop well after this
                            # block's ctxt multiply has drained on DVE
                            pend_s3.append(list(range(4 * j, 4 * j + 4)))
            pend_pv.append((pv_th, hook))

            # 4) fillers: lean while the queue is shallow (pairs 0-1),
            # heavier where the carry work is pushed (pairs 2-3)
            cold = carry is None and bi == 0
            fill(2600 if cold else (380, 520, 580, 580)[p])
            sc_cur = sc_next

    # tail: last block's pending pvs + normalize
    while pend_pv:
        pv_th, hook = pend_pv.popleft()
        pv_th()
        if hook is not None:
            hook()

    if next_carry is not None:
        # next iteration's q/k/v emissions must exist before its stage-2
        # reads them; leftover stage-3 fillers stay queued across the boundary
        run_until(next_carry.pop("mark"))
        next_carry["qtkt"] = next_carry.pop("qtkt_res")[0]
        next_carry["state"] = state
        next_carry["pend_s3"] = pend_s3
        return next_carry

    # push any deferred stage-3 before draining
    for ts in pend_s3:
        for t in ts:
            push(gen_stage3(t))

    # last iteration: drain everything
    while fillers:
        _, th = fillers.popleft()
        th()
        state["popped"] += 1
    return None


def _make_runner(nc):
    import jax
    import jax.core as jcore
    from jax.experimental.shard_map import shard_map
    from jax.sharding import Mesh, NamedSharding, PartitionSpec

    import concourse.mybir as mybir
    from concourse import bass2jax

    bass2jax.install_neuronx_cc_hook()

    in_names, out_names, out_avals, zero_outs = [], [], [], []
    for alloc in nc.m.functions[0].allocations:
        if not isinstance(alloc, mybir.MemoryLocationSet):
            continue
        name = alloc.memorylocations[0].name
        if alloc.kind == "ExternalInput":
            in_names.append(name)
        elif alloc.kind == "ExternalOutput":
            out_names.append(name)
            shape = tuple(alloc.tensor_shape)
            npdt = mybir.dt.np(alloc.dtype)
            out_avals.append(jcore.ShapedArray(shape, npdt))
            zero_outs.append(np.zeros(shape, npdt))
    n_params = len(in_names)
    all_names = tuple(in_names + out_names)

    def _body(*args):
        outs = bass2jax._bass_exec_p.bind(
            *args,
            out_avals=tuple(out_avals),
            in_names=all_names,
            out_names=tuple(out_names),
            lowering_input_output_aliases=(),
            sim_require_finite=True,
            sim_require_nnan=True,
            nc=nc,
        )
        return tuple(outs)

    devices = jax.devices()[:NCORES]
    mesh = Mesh(np.asarray(devices), ("core",))
    nio = n_params + len(out_names)
    sharded = jax.jit(
        shard_map(
            _body, mesh=mesh,
            in_specs=(PartitionSpec("core"),) * nio,
            out_specs=(PartitionSpec("core"),) * len(out_names),
            check_rep=False,
        ),
        keep_unused=True,
    )
    sh = NamedSharding(mesh, PartitionSpec("core"))

    def run(in_maps, reps=1):
        concat = [
            np.concatenate([np.asarray(in_maps[c][n]) for c in range(NCORES)], axis=0)
            for n in in_names
        ]
        concat += [np.concatenate([z] * NCORES, axis=0) for z in zero_outs]
        dev_in = [jax.device_put(a, sh) for a in concat]
        outs = sharded(*dev_in)
        jax.block_until_ready(outs)
        per_call = None
        if reps > 1:
            t0 = time.perf_counter()
            for _ in range(reps - 1):
                outs = sharded(*dev_in)
            jax.block_until_ready(outs)
            per_call = (time.perf_counter() - t0) / (reps - 1)
        results = []
        for c in range(NCORES):
            d = {}
            for i, n in enumerate(out_names):
                arr = np.asarray(outs[i])
                d[n] = arr.reshape((NCORES,) + out_avals[i].shape)[c]
            results.append(d)
        return results, per_call

    return run


_RUNNER = None


def _get_runner():
    global _RUNNER
    if _RUNNER is None:
        from concourse import bass_utils

        nc = _build()

        def run(in_maps, reps=1):
            if reps > 1:
                return _make_runner(nc)(in_maps, reps=reps)
            res = bass_utils.run_bass_kernel_spmd(nc, in_maps, core_ids=list(range(NCORES)))
            return res.results, None

        _RUNNER = run
    return _RUNNER


def _prep_in_maps(x, W_qkv, b_qkv, W_proj):
    in_maps = []
    for core in range(NCORES):
        b, g = core // 2, core % 2
        xT = np.ascontiguousarray(x[b].T)  # [D, S]
        xt = xT.reshape(NDT, 128, NSQ, 512).transpose(2, 1, 0, 3).reshape(NSQ, 128, NDT * 512)
        wq = W_qkv[:, g * 512:(g + 1) * 512]
        wk = W_qkv[:, 1024 + g * 512:1024 + (g + 1) * 512]
        wv = W_qkv[:, 2048 + g * 512:2048 + (g + 1) * 512]
        wqk = np.stack([
            w.reshape(NDT, 128, NPAIR, 128).transpose(2, 1, 0, 3).reshape(NPAIR, 128, NDT * 128)
            for w in (wq, wk)
        ])
        wv_t = wv.reshape(NDT, 128, 512).transpose(1, 0, 2).reshape(128, NDT * 512)
        wp_t = W_proj[g * 512:(g + 1) * 512].reshape(NPAIR, 128, 1024).transpose(1, 0, 2).reshape(128, NPAIR * 1024)
        bq = b_qkv[g * 512:(g + 1) * 512].reshape(NPAIR, 128).T
        bk = b_qkv[1024 + g * 512:1024 + (g + 1) * 512].reshape(NPAIR, 128).T
        bqk = np.concatenate([bq, bk], axis=1)
        in_maps.append({
            "xt": np.ascontiguousarray(xt, np.float16),
            "wqk": np.ascontiguousarray(wqk, np.float16),
            "wv": np.ascontiguousarray(wv_t, np.float16),
            "wp": np.ascontiguousarray(wp_t, np.float16),
            "bqk": np.ascontiguousarray(bqk, np.float32),
        })
    return in_maps


def _assemble(results, b_qkv, W_proj, b_proj):
    const = (b_qkv[2048:3072].astype(np.float64) @ W_proj.astype(np.float64)).astype(np.float32) + b_proj
    parts = [results[c]["out"].reshape(S, D).astype(np.float32) for c in range(NCORES)]
    out = np.stack([parts[2 * b] + parts[2 * b + 1] + const for b in range(B)])
    return out.astype(np.float32)


def kernel(x, W_qkv, b_qkv, W_proj, b_proj, _reps=1):
    x = np.asarray(x, np.float32)
    W_qkv = np.asarray(W_qkv, np.float32)
    b_qkv = np.asarray(b_qkv, np.float32)
    W_proj = np.asarray(W_proj, np.float32)
    b_proj = np.asarray(b_proj, np.float32)
    assert x.shape == (B, S, D), x.shape
    run = _get_runner()
    in_maps = _prep_in_maps(x, W_qkv, b_qkv, W_proj)
    results, per_call = run(in_maps, reps=_reps)
    kernel.last_per_call = per_call
    return _assemble(results, b_qkv, W_proj, b_proj)


kernel.last_per_call = None
